# revision 7
# baseline (speedup 1.0000x reference)
"""Trainium2 Bass kernel for a 5-layer single-token decoder with 15 LM heads.

Sharding (8 cores): tensor-parallel. q/k/v/gate/up sharded on output dim,
o/down on input dim, LM heads sharded on vocab dim. Per layer, the attention
and MLP partial outputs are combined with an AllGather + local sum.

Weights are streamed HBM->SBUF as bf16 (host converts); matmul GEMVs run with
the activation as the 128x1 stationary operand and weights as the moving
operand (1 col/cycle bf16). Small fixups (transposes, softmax, rmsnorm,
reductions) run in f32.
"""
import sys

sys.path.insert(0, "/opt/trn_rl_repo")

import numpy as np

import concourse.bass as bass
import concourse.bacc as bacc
import concourse.mybir as mybir
import concourse.tile as tile
from concourse import bass_utils

F32 = mybir.dt.float32
BF16 = mybir.dt.bfloat16
AX = mybir.AxisListType
OP = mybir.AluOpType
ACT = mybir.ActivationFunctionType

NCORES = 8
H = 1024
NH = 16
NKV = 8
HD = 128
L = 5
S = 16
FF = 4096
NLM = 15
V = 2048
EPS = 1e-6

HC = H // 128          # 8 hidden chunks
QH = NH // NCORES      # 2 q heads per core
FFC = FF // NCORES     # 512 ff per core
VC = V // NCORES       # 256 vocab per core
# per-layer weight stream column offsets (bf16, [128, WCOLS])
QKV_W = QH * HD + HD + HD              # 512
O_OFF = HC * QKV_W                     # 4096
O_W = H                                # per attention-head row-block
GU_OFF = O_OFF + QH * HD // 128 * O_W  # 4096 + 2*1024 = 6144
GU_W = 2 * FFC                         # 1024 (gate 512 | up 512)
D_OFF = GU_OFF + HC * GU_W             # 14336
D_W = H                                # 1024
WCOLS = D_OFF + (FFC // 128) * D_W     # 18432
LM_COLS = NLM * HC * VC                # 30720

_CACHE = {}


def _build_nc():
    nc = bacc.Bacc("TRN2", target_bir_lowering=False, debug=False,
                   enable_asserts=False, num_devices=NCORES)

    def din(name, shape, dt=F32):
        return nc.dram_tensor(name, shape, dt, kind="ExternalInput").ap()

    def dout(name, shape, dt=F32):
        return nc.dram_tensor(name, shape, dt, kind="ExternalOutput").ap()

    ws = din("ws", [L, 128, WCOLS], BF16)
    lmw = din("lmw", [128, LM_COLS], BF16)
    x0 = din("x0", [128, HC])
    kc0 = din("kc0", [128, L * S])
    vc0 = din("vc0", [S, L * HD])
    mb = din("mb", [128, S])          # kv update mask, tiled over partitions
    mt = din("mt", [S, 1])            # kv update mask, [S,1]
    pb = din("pb", [128, S])          # key padding mask, tiled
    lw1 = din("lw1", [128, L * HC])
    lw2 = din("lw2", [128, L * HC])
    nw = din("nw", [128, HC])
    invf = din("invf", [1, HD // 2])
    posb = din("posb", [128, 1])
    eye = din("eye", [128, 128])
    onesc = din("onesc", [128, 1])

    o_logits = dout("logits", [1, NLM * VC])
    o_h = dout("hout", [128, HC])
    o_kc = dout("kcn", [128, L * S])
    o_vc = dout("vcn", [S, L * HD])

    rg = [list(range(NCORES))]

    with tile.TileContext(nc) as tc:
        with tc.tile_pool(name="res", bufs=1) as res, \
             tc.tile_pool(name="wqkv", bufs=2) as pqkv, \
             tc.tile_pool(name="wo", bufs=2) as pwo, \
             tc.tile_pool(name="wgu", bufs=2) as pwgu, \
             tc.tile_pool(name="wd", bufs=2) as pwd, \
             tc.tile_pool(name="act", bufs=2) as pa, \
             tc.tile_pool(name="bigio", bufs=2) as pio, \
             tc.tile_pool(name="hp", bufs=2) as ph, \
             tc.tile_pool(name="ps", bufs=2, space="PSUM") as pp, \
             tc.tile_pool(name="psb", bufs=4, space="PSUM") as ppb, \
             tc.tile_pool(name="dram", bufs=2, space="DRAM") as dr:

            # ---- resident loads ----
            def rtile(shape, src, tag, dt=F32):
                t = res.tile(shape, dt, tag=tag)
                nc.sync.dma_start(t[:], src)
                return t

            sb_x0 = rtile([128, HC], x0[:], "x0")
            sb_kc = rtile([128, L * S], kc0[:], "kc")
            sb_vc = rtile([S, L * HD], vc0[:], "vc")
            sb_mb = rtile([128, S], mb[:], "mb")
            sb_mt = rtile([S, 1], mt[:], "mt")
            sb_pb = rtile([128, S], pb[:], "pb")
            sb_lw1 = rtile([128, L * HC], lw1[:], "lw1")
            sb_lw2 = rtile([128, L * HC], lw2[:], "lw2")
            sb_nw = rtile([128, HC], nw[:], "nw")
            sb_invf = rtile([1, HD // 2], invf[:], "invf")
            sb_posb = rtile([128, 1], posb[:], "posb")
            sb_eye = rtile([128, 128], eye[:], "eye")
            sb_ones = rtile([128, 1], onesc[:], "ones")

            # layer weight stream DMAs (issue all up front; Tile double-buffers
            # via pool slots). Sub-DMA per group so compute can start early.
            wtiles = []
            for i in range(L):
                tq = pqkv.tile([128, O_OFF], BF16, tag="wqkv")
                nc.sync.dma_start(tq[:], ws[i, :, 0:O_OFF])
                to = pwo.tile([128, GU_OFF - O_OFF], BF16, tag="wo")
                nc.sync.dma_start(to[:], ws[i, :, O_OFF:GU_OFF])
                tg = pwgu.tile([128, D_OFF - GU_OFF], BF16, tag="wgu")
                nc.sync.dma_start(tg[:], ws[i, :, GU_OFF:D_OFF])
                td = pwd.tile([128, WCOLS - D_OFF], BF16, tag="wd")
                nc.sync.dma_start(td[:], ws[i, :, D_OFF:WCOLS])
                wtiles.append((tq, to, tg, td))
            sb_lm = res.tile([128, LM_COLS], BF16, tag="lm")
            nc.gpsimd.dma_start(sb_lm[:], lmw[:])

            # ---- setup-derived tensors ----
            sb_im = res.tile([128, S], F32, tag="im")     # 1 - m
            nc.vector.tensor_scalar(out=sb_im[:], in0=sb_mb[:], scalar1=-1.0,
                                    scalar2=1.0, op0=OP.mult, op1=OP.add)
            sb_imt = res.tile([S, 1], F32, tag="imt")
            nc.vector.tensor_scalar(out=sb_imt[:], in0=sb_mt[:], scalar1=-1.0,
                                    scalar2=1.0, op0=OP.mult, op1=OP.add)
            sb_padx = res.tile([128, S], F32, tag="padx")  # pad * sqrt(HD)
            nc.vector.tensor_scalar_mul(sb_padx[:], sb_pb[:], float(np.sqrt(HD)))

            # small float constants for activation biases
            c_eps = res.tile([1, 1], F32, tag="ceps")
            nc.vector.memset(c_eps[:], float(EPS))

            # rope tables, free layout [1, 64] -> tripled [1, 192]
            sb_freq = res.tile([1, HD // 2], F32, tag="freq")
            nc.vector.tensor_scalar_mul(sb_freq[:], sb_invf[:], sb_posb[0:1, :])
            # range-reduce into [-pi, pi] (two wraps cover pos*invf in [0, 5pi))
            PI = float(np.pi)
            wr_s = res.tile([1, HD // 2], F32, tag="wrs")
            nc.vector.add_range_wrap(wr_s[:], sb_freq[:], 0.0, PI, 2 * PI)
            nc.vector.add_range_wrap(wr_s[:], wr_s[:], 0.0, PI, 2 * PI)
            wr_c = res.tile([1, HD // 2], F32, tag="wrc")
            nc.vector.add_range_wrap(wr_c[:], sb_freq[:], PI / 2, PI, 2 * PI)
            nc.vector.add_range_wrap(wr_c[:], wr_c[:], 0.0, PI, 2 * PI)
            sb_cs = res.tile([1, 3 * HD // 2], F32, tag="cs")
            sb_sn = res.tile([1, 3 * HD // 2], F32, tag="sn")
            for j in range(3):
                nc.scalar.activation(sb_cs[0:1, j * 64:(j + 1) * 64], wr_c[:], ACT.Sin)
                nc.scalar.activation(sb_sn[0:1, j * 64:(j + 1) * 64], wr_s[:], ACT.Sin)

            # output caches (also serve as the attention k/v source)
            sb_kcn = res.tile([128, L * S], F32, tag="kcn")
            sb_vcn = res.tile([S, L * HD], F32, tag="vcn")

            def rmsnorm(h_t, wcols, out_bf):
                sq = pa.tile([128, HC], F32, tag="sq")
                ssq = pa.tile([128, 1], F32, tag="ssq")
                nc.scalar.activation(sq[:], h_t[:], ACT.Square, accum_out=ssq[:])
                tot = ppb.tile([1, 1], F32, tag="psml")
                nc.tensor.matmul(tot[:], ssq[:], sb_ones[:], start=True, stop=True)
                srt = pa.tile([1, 1], F32, tag="srt")
                nc.scalar.activation(srt[:], tot[:], ACT.Sqrt,
                                     scale=float(1.0 / H), bias=c_eps[:])
                rin = pa.tile([1, 1], F32, tag="rin")
                nc.vector.reciprocal(rin[:], srt[:])
                rb = pa.tile([128, 1], F32, tag="rb")
                nc.gpsimd.partition_broadcast(rb[:], rin[:])
                nc.vector.scalar_tensor_tensor(
                    out=out_bf[:], in0=h_t[:], scalar=rb[:], in1=wcols,
                    op0=OP.mult, op1=OP.mult)

            def ag_sum_res(partial_sb, h_prev, tag):
                """AllGather the [1,1024] partial, sum 8 rows + h_prev -> new h."""
                bin_ = dr.tile([1, H], F32, tag=tag + "_i")
                nc.sync.dma_start(bin_[:], partial_sb[:])
                bout = dr.tile([NCORES, H], F32, tag=tag + "_o")
                nc.gpsimd.collective_compute(
                    "AllGather", OP.bypass, replica_groups=rg,
                    ins=[bin_.opt()], outs=[bout.opt()])
                ag = pio.tile([NCORES, H], F32, tag="bigio")
                nc.sync.dma_start(ag[:], bout[:])
                pr = ppb.tile([128, HC], F32, tag="psml")
                for c in range(HC):
                    nc.tensor.matmul(pr[:, c:c + 1],
                                     ag[0:NCORES, c * 128:(c + 1) * 128],
                                     sb_ones[0:NCORES, :], start=True, stop=True)
                hn = ph.tile([128, HC], F32, tag="h")
                nc.vector.tensor_add(hn[:], pr[:], h_prev[:])
                return hn

            h = sb_x0
            for i in range(L):
                wq, wo, wg, wd = wtiles[i]
                # ---- ln1 + qkv ----
                xb = pa.tile([128, HC], BF16, tag="xb")
                rmsnorm(h, sb_lw1[:, i * HC:(i + 1) * HC], xb)
                pqkv_ps = pp.tile([1, QKV_W], F32, tag="pbig")
                for k in range(HC):
                    nc.tensor.matmul(pqkv_ps[:], xb[:, k:k + 1],
                                     wq[:, k * QKV_W:(k + 1) * QKV_W],
                                     start=(k == 0), stop=(k == HC - 1))
                qkv = pa.tile([1, QKV_W], F32, tag="mid")
                nc.vector.tensor_copy(qkv[:], pqkv_ps[:])

                # ---- rope on q0,q1,k (free layout, batched via 3D APs) ----
                def lohi(t, off):
                    return t[0:1, :].rearrange("p (j d) -> p j d", d=HD)[:, 0:3, off:off + 64]

                c3 = sb_cs[0:1, :].rearrange("p (j d) -> p j d", d=64)
                s3 = sb_sn[0:1, :].rearrange("p (j d) -> p j d", d=64)
                t1 = pa.tile([1, 192], F32, tag="t1")
                t2 = pa.tile([1, 192], F32, tag="t2")
                rq = pa.tile([1, 3 * HD], F32, tag="rq")
                v1 = t1[0:1, :].rearrange("p (j d) -> p j d", d=64)
                v2 = t2[0:1, :].rearrange("p (j d) -> p j d", d=64)
                # lo' = lo*cos - hi*sin ; hi' = hi*cos + lo*sin
                nc.vector.tensor_mul(v1, lohi(qkv, 64), s3)
                nc.vector.tensor_mul(v2, lohi(qkv, 0), c3)
                nc.vector.tensor_sub(lohi(rq, 0), v2, v1)
                nc.vector.tensor_mul(v1, lohi(qkv, 0), s3)
                nc.vector.tensor_mul(v2, lohi(qkv, 64), c3)
                nc.vector.tensor_add(lohi(rq, 64), v2, v1)

                # ---- transpose q0,q1,k -> [128, 3] ----
                ptq = ppb.tile([128, 3], F32, tag="psml")
                for j in range(3):
                    nc.tensor.transpose(ptq[:, j:j + 1],
                                        rq[0:1, j * HD:(j + 1) * HD],
                                        sb_eye[0:1, 0:1])
                tq = pa.tile([128, 3], F32, tag="tq")
                nc.vector.tensor_copy(tq[:], ptq[:])

                # ---- kv cache update ----
                kslice = sb_kcn[:, i * S:(i + 1) * S]
                tkc = pa.tile([128, S], F32, tag="tkc")
                nc.vector.tensor_mul(tkc[:], sb_kc[:, i * S:(i + 1) * S], sb_im[:])
                nc.vector.scalar_tensor_tensor(
                    out=kslice, in0=sb_mb[:], scalar=tq[:, 2:3], in1=tkc[:],
                    op0=OP.mult, op1=OP.add)
                vslice = sb_vcn[:, i * HD:(i + 1) * HD]
                tvc = pa.tile([S, HD], F32, tag="tvc")
                nc.vector.tensor_scalar_mul(tvc[:], sb_vc[:, i * HD:(i + 1) * HD],
                                            sb_imt[:])
                pvm = ppb.tile([S, HD], F32, tag="psml")
                nc.tensor.matmul(pvm[:], sb_mb[0:1, :],
                                 qkv[0:1, (QH + 1) * HD:(QH + 2) * HD],
                                 start=True, stop=True)
                nc.vector.tensor_add(vslice, pvm[:], tvc[:])

                # ---- scores + softmax ----
                psc = ppb.tile([QH, S], F32, tag="psml")
                nc.tensor.matmul(psc[:], tq[:, 0:QH], kslice, start=True, stop=True)
                sc = pa.tile([QH, S], F32, tag="sc")
                nc.vector.tensor_add(sc[:], psc[:], sb_padx[0:QH, :])
                ex = pa.tile([QH, S], F32, tag="ex")
                rsum = pa.tile([QH, 1], F32, tag="rsum")
                nc.scalar.activation(ex[:], sc[:], ACT.Exp,
                                     scale=float(1.0 / np.sqrt(HD)),
                                     accum_out=rsum[:])
                rrec = pa.tile([QH, 1], F32, tag="rrec")
                nc.vector.reciprocal(rrec[:], rsum[:])
                w2 = pa.tile([QH, S], F32, tag="w2")
                nc.scalar.activation(w2[:], ex[:], ACT.Copy, scale=rrec[:])

                # ---- attn = vc^T @ w^T -> [128, QH] ----
                pwt = ppb.tile([S, QH], F32, tag="psml")
                nc.tensor.transpose(pwt[:], w2[:], sb_eye[0:QH, 0:QH])
                wt = pa.tile([S, QH], F32, tag="wt")
                nc.vector.tensor_copy(wt[:], pwt[:])
                pav = ppb.tile([128, QH], F32, tag="psml")
                nc.tensor.matmul(pav[:], vslice, wt[:], start=True, stop=True)
                av = pa.tile([128, QH], BF16, tag="av")
                nc.vector.tensor_copy(av[:], pav[:])

                # ---- o-proj partial [1, 1024] ----
                po = pp.tile([1, H], F32, tag="pbig")
                for hh in range(QH):
                    for half in range(2):
                        nc.tensor.matmul(
                            po[0:1, half * 512:(half + 1) * 512],
                            av[:, hh:hh + 1],
                            wo[:, hh * O_W + half * 512: hh * O_W + (half + 1) * 512],
                            start=(hh == 0), stop=(hh == QH - 1))
                posb_t = pio.tile([1, H], F32, tag="bigio")
                nc.vector.tensor_copy(posb_t[:], po[:])
                h = ag_sum_res(posb_t, h, f"at{i}")

                # ---- ln2 + mlp ----
                xb2 = pa.tile([128, HC], BF16, tag="xb2")
                rmsnorm(h, sb_lw2[:, i * HC:(i + 1) * HC], xb2)
                pg = pp.tile([1, FFC], F32, tag="pbig")
                pu = pp.tile([1, FFC], F32, tag="pbig")
                for k in range(HC):
                    nc.tensor.matmul(pg[:], xb2[:, k:k + 1],
                                     wg[:, k * GU_W:k * GU_W + FFC],
                                     start=(k == 0), stop=(k == HC - 1))
                    nc.tensor.matmul(pu[:], xb2[:, k:k + 1],
                                     wg[:, k * GU_W + FFC:(k + 1) * GU_W],
                                     start=(k == 0), stop=(k == HC - 1))
                sg = pa.tile([1, FFC], F32, tag="mid")
                nc.scalar.activation(sg[:], pg[:], ACT.Sigmoid)
                gsg = pa.tile([1, FFC], F32, tag="mid2")
                nc.vector.tensor_mul(gsg[:], sg[:], pg[:])
                gu = pa.tile([1, FFC], F32, tag="mid")
                nc.vector.tensor_mul(gu[:], gsg[:], pu[:])
                pgt = ppb.tile([128, FFC // 128], F32, tag="psml")
                for j in range(FFC // 128):
                    nc.tensor.transpose(pgt[:, j:j + 1],
                                        gu[0:1, j * 128:(j + 1) * 128],
                                        sb_eye[0:1, 0:1])
                gut = pa.tile([128, FFC // 128], BF16, tag="gut")
                nc.vector.tensor_copy(gut[:], pgt[:])
                pd = pp.tile([1, H], F32, tag="pbig")
                for c4 in range(FFC // 128):
                    for half in range(2):
                        nc.tensor.matmul(
                            pd[0:1, half * 512:(half + 1) * 512],
                            gut[:, c4:c4 + 1],
                            wd[:, c4 * D_W + half * 512: c4 * D_W + (half + 1) * 512],
                            start=(c4 == 0), stop=(c4 == FFC // 128 - 1))
                pdsb = pio.tile([1, H], F32, tag="bigio")
                nc.vector.tensor_copy(pdsb[:], pd[:])
                h = ag_sum_res(pdsb, h, f"mlp{i}")

            # ---- final norm + LM heads ----
            xf = pa.tile([128, HC], BF16, tag="xf")
            rmsnorm(h, sb_nw[:], xf)
            sb_logits = res.tile([1, NLM * VC], F32, tag="logits")
            for n in range(NLM):
                pl = ppb.tile([1, VC], F32, tag="psml")
                for k in range(HC):
                    nc.tensor.matmul(pl[:], xf[:, k:k + 1],
                                     sb_lm[:, n * (HC * VC) + k * VC:
                                           n * (HC * VC) + (k + 1) * VC],
                                     start=(k == 0), stop=(k == HC - 1))
                nc.vector.tensor_copy(sb_logits[0:1, n * VC:(n + 1) * VC], pl[:])

            # ---- outputs ----
            nc.sync.dma_start(o_logits[:], sb_logits[:])
            nc.sync.dma_start(o_h[:], h[:])
            nc.sync.dma_start(o_kc[:], sb_kcn[:])
            nc.sync.dma_start(o_vc[:], sb_vcn[:])

    nc.finalize()
    return nc


def _get_nc():
    if "nc" not in _CACHE:
        _CACHE["nc"] = _build_nc()
    return _CACHE["nc"]


def shard_inputs(inputs):
    import ml_dtypes
    bf16 = ml_dtypes.bfloat16
    f32 = np.float32

    Wq = np.asarray(inputs["Wq"], f32)
    Wk = np.asarray(inputs["Wk"], f32)
    Wv = np.asarray(inputs["Wv"], f32)
    Wo = np.asarray(inputs["Wo"], f32)
    Wg = np.asarray(inputs["Wg"], f32)
    Wu = np.asarray(inputs["Wu"], f32)
    Wd = np.asarray(inputs["Wd"], f32)
    lm_w = np.asarray(inputs["lm_w"], f32)
    K0 = np.asarray(inputs["key_cache"], f32)[0, :, 0, :]      # [5120, 16]
    V0 = np.asarray(inputs["value_cache"], f32)[0, :, 0, :]
    m = np.asarray(inputs["kv_cache_update_mask"], f32)[0]     # [16]
    pad = np.asarray(inputs["key_padding_mask"], f32)[0]       # [16]
    ln1 = np.asarray(inputs["ln1_w"], f32)
    ln2 = np.asarray(inputs["ln2_w"], f32)
    nwv = np.asarray(inputs["norm_w"], f32)
    invf = np.asarray(inputs["inv_freq"], f32)
    poslen = np.asarray(inputs["cache_length"], f32)
    x = np.asarray(inputs["input_embeds"], f32).reshape(H)

    x0 = x.reshape(HC, 128).T.copy()                            # [128, 8]
    lw1 = np.concatenate([ln1[i].reshape(HC, 128).T for i in range(L)], axis=1)
    lw2 = np.concatenate([ln2[i].reshape(HC, 128).T for i in range(L)], axis=1)
    nw = nwv.reshape(HC, 128).T.copy()
    mb = np.tile(m.reshape(1, S), (128, 1))
    mt = m.reshape(S, 1).copy()
    pb = np.tile(pad.reshape(1, S), (128, 1))
    eye = np.eye(128, dtype=f32)
    onesc = np.ones((128, 1), f32)
    posb = np.full((128, 1), poslen[0], f32)
    invf_r = invf.reshape(1, HD // 2)

    in_maps = []
    for c in range(NCORES):
        ws = np.empty((L, 128, WCOLS), bf16)
        for i in range(L):
            for k in range(HC):
                r = slice(k * 128, (k + 1) * 128)
                col = k * QKV_W
                ws[i, :, col:col + QH * HD] = Wq[i][r, c * QH * HD:(c + 1) * QH * HD]
                ws[i, :, col + QH * HD:col + (QH + 1) * HD] = Wk[i][r, c * HD:(c + 1) * HD]
                ws[i, :, col + (QH + 1) * HD:col + QKV_W] = Wv[i][r, c * HD:(c + 1) * HD]
            for hh in range(QH):
                rr = slice(c * QH * HD + hh * HD, c * QH * HD + (hh + 1) * HD)
                ws[i, :, O_OFF + hh * O_W:O_OFF + (hh + 1) * O_W] = Wo[i][rr, :]
            for k in range(HC):
                r = slice(k * 128, (k + 1) * 128)
                col = GU_OFF + k * GU_W
                ws[i, :, col:col + FFC] = Wg[i][r, c * FFC:(c + 1) * FFC]
                ws[i, :, col + FFC:col + 2 * FFC] = Wu[i][r, c * FFC:(c + 1) * FFC]
            for j in range(FFC // 128):
                rr = slice(c * FFC + j * 128, c * FFC + (j + 1) * 128)
                ws[i, :, D_OFF + j * D_W:D_OFF + (j + 1) * D_W] = Wd[i][rr, :]
        lmw = np.empty((128, LM_COLS), bf16)
        for n in range(NLM):
            for k in range(HC):
                lmw[:, n * (HC * VC) + k * VC:n * (HC * VC) + (k + 1) * VC] = \
                    lm_w[n, k * 128:(k + 1) * 128, c * VC:(c + 1) * VC]
        kc0 = np.concatenate(
            [K0[i * H + c * HD:i * H + (c + 1) * HD, :] for i in range(L)], axis=1)
        vc0 = np.concatenate(
            [V0[i * H + c * HD:i * H + (c + 1) * HD, :].T for i in range(L)], axis=1)
        in_maps.append({
            "ws": ws, "lmw": lmw, "x0": x0, "kc0": np.ascontiguousarray(kc0),
            "vc0": np.ascontiguousarray(vc0), "mb": mb, "mt": mt, "pb": pb,
            "lw1": lw1, "lw2": lw2, "nw": nw, "invf": invf_r, "posb": posb,
            "eye": eye, "onesc": onesc,
        })
    return in_maps


def assemble_outputs(results):
    f32 = np.float32
    logits = np.zeros((1, NLM, V), f32)
    for c in range(NCORES):
        lg = np.asarray(results[c]["logits"], f32).reshape(NLM, VC)
        logits[0, :, c * VC:(c + 1) * VC] = lg
    hout = np.asarray(results[0]["hout"], f32)
    hidden = hout.T.reshape(1, H, 1, 1).astype(f32)
    new_key = np.zeros((1, L * NKV * HD, 1, S), f32)
    new_val = np.zeros((1, L * NKV * HD, 1, S), f32)
    for c in range(NCORES):
        kcn = np.asarray(results[c]["kcn"], f32)       # [128, L*S]
        vcn = np.asarray(results[c]["vcn"], f32)       # [S, L*HD]
        for i in range(L):
            rows = slice(i * H + c * HD, i * H + (c + 1) * HD)
            new_key[0, rows, 0, :] = kcn[:, i * S:(i + 1) * S]
            new_val[0, rows, 0, :] = vcn[:, i * HD:(i + 1) * HD].T
    return logits, hidden, new_key, new_val


def kernel(**inputs):
    nc = _get_nc()
    in_maps = shard_inputs(inputs)
    res = bass_utils.run_bass_kernel_spmd(
        nc, in_maps, core_ids=list(range(NCORES)))
    return assemble_outputs(res.results)
